# revision 1
# baseline (speedup 1.0000x reference)
"""Trainium2 Bass kernel for the CSTR (evaporator) 1M-step scan.

Parallel-in-time: the per-step map is contractive (~0.965/step slow mode),
so the trajectory is split into 1024 segments (8 cores x 128 lanes) of
L=1024 steps, each extended K=384 steps back ("spin-up") so an arbitrary
segment-entry state converges below fp32 noise before the graded region.
Within each lane's window the nonlinear recurrence

  x0' = x0*(SC(u0) - c02*x0 - c03*x1) + c01
  x1' = SA(u1)*x1 + a10*x0 + SB(u0,u1)

is solved by 3 Picard-Gauss-Seidel sweeps whose linear-recurrence cores
run on the vector engine's native tensor_tensor_scan. Later sweeps start
at column 64/224: contraction washes the inherited suffix. Input DMA is
split into 2 column chunks on two DGE queues with stream precompute and
the first sweep chasing the chunks. The first K outputs are computed on
host (0.1% of the work) since segment 0 has no spin-up protection.
All param-derived scalars are per-partition [128,1] operands, so the
compiled program is input-independent.
"""

import numpy as np

T = 1048576
P = 128
NCORES = 8
L = 1024          # graded steps per lane
K = 384           # spin-up steps
W = K + L         # window length per lane (1408)
TC = T // NCORES  # steps per core
SLAB = TC + K     # u rows staged per core
NSWEEPS = 3
SWEEP_J0 = [0, 64, 224]
NQ = 2            # head chunks
NC_CONST = 17

# fixed model constants (match reference.py)
A, B, C_, D, E, F_, G, H = 0.5616, 0.3126, 48.43, 0.507, 55.0, 0.1538, 90.0, 0.16

_cache = {}


def _build_nc():
    if "nc" in _cache:
        return _cache["nc"]
    from contextlib import ExitStack
    import concourse.bacc as bacc
    import concourse.tile as tile
    import concourse.mybir as mybir
    from bass_rust import AP

    f32 = mybir.dt.float32
    op = mybir.AluOpType
    ident = mybir.ActivationFunctionType.Identity
    nc = bacc.Bacc("TRN2", target_bir_lowering=False, debug=False,
                   enable_asserts=True, num_devices=NCORES)

    uslab = nc.dram_tensor("uslab", [SLAB, 2], f32, kind="ExternalInput").ap()
    cons = nc.dram_tensor("cons", [P, NC_CONST], f32, kind="ExternalInput").ap()
    o0 = nc.dram_tensor("o0", [P, L], f32, kind="ExternalOutput").ap()
    o1 = nc.dram_tensor("o1", [P, L], f32, kind="ExternalOutput").ap()

    Wm = W - 1
    CHUNKS = [(0, W // 2), (W // 2, W)]

    with tile.TileContext(nc) as tc, ExitStack() as ctx:
        pool = ctx.enter_context(tc.tile_pool(name="main", bufs=1))
        t_uwq = [pool.tile([P, 2 * (hi - lo)], f32, name=f"uw{q}", tag=f"uw{q}")
                 for q, (lo, hi) in enumerate(CHUNKS)]
        t_cons = pool.tile([P, NC_CONST], f32, tag="cons")

        def cst(i):
            return t_cons[:, i : i + 1]

        t_scr = pool.tile([P, W], f32, tag="scr")   # recip scratch
        t_rec = pool.tile([P, W], f32, tag="rec")
        t_den = pool.tile([P, W], f32, tag="den")
        t_r = pool.tile([P, W], f32, tag="r")
        t_SA = pool.tile([P, W], f32, tag="SA")
        t_SBp = pool.tile([P, W], f32, tag="SBp")
        t_SB = pool.tile([P, W], f32, tag="SB")
        t_SC = pool.tile([P, W], f32, tag="SC")
        t_b = pool.tile([P, W], f32, tag="b")
        t_v = pool.tile([P, Wm], f32, tag="v")
        t_a = pool.tile([P, Wm], f32, tag="a")
        t_c = pool.tile([P, Wm], f32, tag="c")
        t_X0 = pool.tile([P, W], f32, tag="X0")
        t_X1 = pool.tile([P, W], f32, tag="X1")

        nc.sync.dma_start(t_cons[:], cons[:])
        # warm both HWDGE queues (first-use ring init costs ~2.3us)
        nc.sync.dma_start(t_scr[0:1, 0:4], cons[0:1, 0:4])
        nc.scalar.dma_start(t_scr[0:1, 4:8], cons[0:1, 0:4])
        # ACT table warm-up (Identity) while DMA streams in
        nc.scalar.activation(t_scr[:, 0:1], t_cons[:, 0:1], ident, bias=0.0, scale=1.0)

        # input windows: 4 column chunks, each its own tile (fine-grained
        # deps so compute chases the DMA), alternating two DGE queues
        for q, (lo, hi) in enumerate(CHUNKS):
            eng = nc.sync if q % 2 == 0 else nc.scalar
            winq = AP(uslab.tensor, 2 * lo, [[L * 2, P], [1, 2 * (hi - lo)]])
            eng.dma_start(t_uwq[q][:], winq)

        nc.vector.tensor_copy(t_X0[:, 0:1], cst(15))
        nc.vector.tensor_copy(t_X1[:, 0:1], cst(16))

        # per chunk: stream precompute then the chunk's sweep-1 piece
        for q, (lo, hi) in enumerate(CHUNKS):
            wq = hi - lo
            u0q = t_uwq[q][:, 0 : 2 * wq : 2]
            u1q = t_uwq[q][:, 1 : 2 * wq : 2]
            nc.vector.tensor_scalar(t_den[:, lo:hi], u1q, cst(0), cst(1), op.mult, op.add)
            nc.vector.reciprocal_approx_fast(t_rec[:, lo:hi], t_den[:, lo:hi])
            nc.vector.scalar_tensor_tensor(t_r[:, lo:hi], u1q, cst(2),
                                           t_rec[:, lo:hi], op.mult, op.mult)
            nc.scalar.activation(t_SC[:, lo:hi], u0q, ident, bias=cst(9), scale=cst(8))
            nc.scalar.activation(t_SBp[:, lo:hi], u0q, ident, bias=cst(6), scale=cst(5))
            nc.scalar.activation(t_SA[:, lo:hi], t_r[:, lo:hi], ident,
                                 bias=cst(4), scale=cst(3))
            nc.scalar.activation(t_b[:, lo:hi], t_den[:, lo:hi], ident,
                                 bias=cst(11), scale=0.0)
            nc.vector.scalar_tensor_tensor(t_SB[:, lo:hi], t_r[:, lo:hi], cst(7),
                                           t_SBp[:, lo:hi], op.mult, op.add)
            # sweep-1 pieces for this chunk
            shi = min(hi, Wm)
            nc.vector.tensor_scalar(t_a[:, lo:shi], t_SC[:, lo:shi], cst(10),
                                    None, op.subtract)
            nc.vector.tensor_tensor_scan(t_X0[:, lo + 1 : shi + 1], t_a[:, lo:shi],
                                         t_b[:, lo:shi], t_X0[:, lo : lo + 1],
                                         op.mult, op.add)
            nc.vector.scalar_tensor_tensor(t_c[:, lo:shi], t_X0[:, lo:shi], cst(12),
                                           t_SB[:, lo:shi], op.mult, op.add)
            nc.vector.tensor_tensor_scan(t_X1[:, lo + 1 : shi + 1], t_SA[:, lo:shi],
                                         t_c[:, lo:shi], t_X1[:, lo : lo + 1],
                                         op.mult, op.add)

        # sweeps 2..N-1: full-range single ops
        for s in range(1, NSWEEPS - 1):
            j0 = SWEEP_J0[s]
            nc.vector.scalar_tensor_tensor(t_v[:, j0:Wm], t_X0[:, j0:Wm], cst(13),
                                           t_SC[:, j0:Wm], op.mult, op.add)
            nc.vector.scalar_tensor_tensor(t_a[:, j0:Wm], t_X1[:, j0:Wm], cst(14),
                                           t_v[:, j0:Wm], op.mult, op.add)
            nc.vector.tensor_tensor_scan(t_X0[:, j0 + 1 : W], t_a[:, j0:Wm],
                                         t_b[:, j0:Wm], t_X0[:, j0 : j0 + 1],
                                         op.mult, op.add)
            nc.vector.scalar_tensor_tensor(t_c[:, j0:Wm], t_X0[:, j0:Wm], cst(12),
                                           t_SB[:, j0:Wm], op.mult, op.add)
            nc.vector.tensor_tensor_scan(t_X1[:, j0 + 1 : W], t_SA[:, j0:Wm],
                                         t_c[:, j0:Wm], t_X1[:, j0 : j0 + 1],
                                         op.mult, op.add)

        # final sweep: chunked x3, output DMA inline (short tail)
        j0 = SWEEP_J0[NSWEEPS - 1]
        bounds = [(j0, 760), (760, 1180), (1180, Wm)]
        nc.vector.scalar_tensor_tensor(t_v[:, j0:Wm], t_X0[:, j0:Wm], cst(13),
                                       t_SC[:, j0:Wm], op.mult, op.add)
        nc.vector.scalar_tensor_tensor(t_a[:, j0:Wm], t_X1[:, j0:Wm], cst(14),
                                       t_v[:, j0:Wm], op.mult, op.add)
        for i, (lo, hi) in enumerate(bounds):
            nc.vector.tensor_tensor_scan(t_X0[:, lo + 1 : hi + 1], t_a[:, lo:hi],
                                         t_b[:, lo:hi], t_X0[:, lo : lo + 1],
                                         op.mult, op.add)
            nc.vector.scalar_tensor_tensor(t_c[:, lo:hi], t_X0[:, lo:hi], cst(12),
                                           t_SB[:, lo:hi], op.mult, op.add)
            dlo, dhi = max(lo + 1, K), hi + 1
            if dhi > dlo:
                nc.sync.dma_start(o0[:, dlo - K : dhi - K], t_X0[:, dlo:dhi])
            nc.vector.tensor_tensor_scan(t_X1[:, lo + 1 : hi + 1], t_SA[:, lo:hi],
                                         t_c[:, lo:hi], t_X1[:, lo : lo + 1],
                                         op.mult, op.add)
            if dhi > dlo:
                nc.scalar.dma_start(o1[:, dlo - K : dhi - K], t_X1[:, dlo:dhi])

    nc.compile()
    _cache["nc"] = nc
    return nc


def _derive(params, x0):
    M, Cc, UA2, Cp, lam, lams, F1, X1p, F3, T1, T200 = [float(params[i]) for i in range(11)]
    UA1 = H * (F1 + F3)
    k1 = (UA1 + F1 * Cp) / lam
    p_ = k1 * B
    q_ = k1 * A
    alpha_u = UA1 * F_ / lam
    alpha_c = (UA1 * G + F1 * Cp * T1) / lam - k1 * C_
    c01 = F1 * X1p / M
    c02 = p_ / M
    c03 = q_ / M
    a10 = -p_ / Cc
    i0, i1 = float(x0[0]), float(x0[1])

    cv = np.zeros(NC_CONST, np.float64)
    cv[0] = 2.0 * Cp
    cv[1] = UA2
    cv[2] = 2.0 * Cp * UA2
    cv[3] = -D / (lam * Cc)               # cA2
    cv[4] = 1.0 - q_ / Cc                 # cA1
    cv[5] = alpha_u / Cc                  # cB2
    cv[6] = alpha_c / Cc                  # cB1
    cv[7] = -(E - T200) / (lam * Cc)      # cB3
    cv[8] = alpha_u / M                   # cC2
    cv[9] = 1.0 - (F1 - alpha_c) / M      # cC1
    cv[10] = c02 * i0 + c03 * i1          # sweep-1 a offset
    cv[11] = c01                          # scan0 additive const
    cv[12] = a10                          # c coefficient
    cv[13] = -c02
    cv[14] = -c03
    cv[15] = i0
    cv[16] = i1
    return cv.astype(np.float32)


def _make_in_maps(u, x0, params):
    u = np.ascontiguousarray(u, np.float32)
    cons = np.tile(_derive(params, x0)[None, :], (P, 1))
    in_maps = []
    for c in range(NCORES):
        if c == 0:
            slab = np.concatenate([np.repeat(u[0:1], K, axis=0), u[0:TC]], axis=0)
        else:
            slab = u[c * TC - K : c * TC + TC]
        in_maps.append({
            "uslab": np.ascontiguousarray(slab),
            "cons": cons,
        })
    return in_maps


def _host_head(u, x0, params, n):
    # exact fp32 simulation of the first n steps (segment 0 has no spin-up)
    f = np.float32
    M, Cc, UA2, Cp, lam, lams, F1, X1p, F3, T1, T200 = [f(params[i]) for i in range(11)]
    out = np.empty((n, 2), f)
    s0, s1 = f(x0[0]), f(x0[1])
    fA, fB, fC, fD, fE, fF, fG, fH = f(A), f(B), f(C_), f(D), f(E), f(F_), f(G), f(H)
    one, two = f(1.0), f(2.0)
    UA1 = fH * (F1 + F3)
    for t in range(n):
        out[t, 0] = s0
        out[t, 1] = s1
        u0, u1 = f(u[t, 0]), f(u[t, 1])
        T2 = fA * s1 + fB * s0 + fC
        T3 = fD * s1 + fE
        T100 = fF * u0 + fG
        Q100 = UA1 * (T100 - T2)
        Q200 = UA2 * (T3 - T200) / (one + UA2 / (two * Cp * u1))
        F5 = Q200 / lam
        F4 = (Q100 - F1 * Cp * (T2 - T1)) / lam
        F2 = F1 - F4
        X2d = (F1 * X1p - F2 * s0) / M
        P2d = (F4 - F5) / Cc
        s0 = s0 + X2d
        s1 = s1 + P2d
    return out


def _assemble(results, head):
    out = np.empty((T, 2), np.float32)
    for c in range(NCORES):
        out[c * TC : (c + 1) * TC, 0] = results[c]["o0"].reshape(-1)
        out[c * TC : (c + 1) * TC, 1] = results[c]["o1"].reshape(-1)
    out[0:L] = head
    return out


def run(u_forced, x0, params, trace=False):
    from concourse.bass_utils import run_bass_kernel_spmd
    nc = _build_nc()
    in_maps = _make_in_maps(u_forced, x0, params)
    head = _host_head(u_forced, x0, params, L)
    res = run_bass_kernel_spmd(nc, in_maps, list(range(NCORES)), trace=trace)
    return _assemble(res.results, head), res


def kernel(u_forced, x0, params):
    out, _ = run(u_forced, x0, params, trace=False)
    return out



# revision 4
# speedup vs baseline: 1.3673x; 1.3673x over previous
"""Trainium2 Bass kernel for the CSTR (evaporator) 1M-step scan.

Parallel-in-time: the per-step map is contractive (slow mode ~0.9665/step),
so the trajectory is split into 1024 segments (8 cores x 128 lanes) of
L=1024 steps, each extended K=192 steps back ("spin-up") so an arbitrary
segment-entry state converges below tolerance before the graded region.
Within each lane's window the nonlinear recurrence

  x0' = x0*(SC(u0) - c02*x0 - c03*x1) + c01
  x1' = SA(u1)*x1 + a10*x0 + SB(u0,u1)

is solved by 2 Picard-Gauss-Seidel sweeps (second sweep re-scans from
column 64). States are rescaled (Y0 = x0/c01, Y1 = x1/(a10*c01)) so the
X0-scan additive term is the constant 1.0 and the c-links become pure
tensor-tensor ADDs: the vector engine runs only the 4 linear-recurrence
scans (tensor_tensor_scan), the reciprocal, and the two sweep-2
coefficient links; all affine precompute runs on the scalar (ACT) engine
and the c/SB links run on the gpsimd (Pool) engine as tensor_tensor
add/sub, pipelined in 4 column chunks so the scans run back-to-back.
Input DMA is chunked on two DGE queues with a small first chunk so the
first scan starts early; outputs stream out per chunk. The first L rows
are computed on host (segment 0 has no spin-up). All param-derived
scalars are per-partition [128,1] operands, so the compiled program is
input-independent; outputs are unscaled on host.
"""

import numpy as np

T = 1048576
P = 128
NCORES = 8
L = 1024          # graded steps per lane
K = 192           # spin-up steps
W = K + L         # window length per lane (1216)
J0 = 64           # sweep-2 restart column
TC = T // NCORES  # steps per core
SLAB = TC + K     # u rows staged per core
NC_CONST = 17

# fixed model constants (match reference.py)
A, B, C_, D, E, F_, G, H = 0.5616, 0.3126, 48.43, 0.507, 55.0, 0.1538, 90.0, 0.16

# column chunking
CH_DMA = [(0, 192), (192, 704), (704, 1088), (1088, 1216)]
CH_S1 = [(0, 192), (192, 704), (704, 1088), (1088, 1215)]
CH_S2 = [(64, 192), (192, 704), (704, 1088), (1088, 1215)]
# X-column ranges streamed to output after sweep-2 scan chunks 1,2,3
OUT_CH = [(192, 705), (705, 1089), (1089, 1216)]

_cache = {}


def _build_nc():
    if "nc" in _cache:
        return _cache["nc"]
    from contextlib import ExitStack
    import concourse.bacc as bacc
    import concourse.tile as tile
    import concourse.mybir as mybir
    from bass_rust import AP

    f32 = mybir.dt.float32
    op = mybir.AluOpType
    ident = mybir.ActivationFunctionType.Identity
    nc = bacc.Bacc("TRN2", target_bir_lowering=False, debug=False,
                   enable_asserts=True, num_devices=NCORES)

    uslab = nc.dram_tensor("uslab", [SLAB, 2], f32, kind="ExternalInput").ap()
    cons = nc.dram_tensor("cons", [P, NC_CONST], f32, kind="ExternalInput").ap()
    o0 = nc.dram_tensor("o0", [P, L], f32, kind="ExternalOutput").ap()
    o1 = nc.dram_tensor("o1", [P, L], f32, kind="ExternalOutput").ap()

    with tile.TileContext(nc) as tc, ExitStack() as ctx:
        pool = ctx.enter_context(tc.tile_pool(name="main", bufs=1))
        t_u = [pool.tile([P, 2 * (hi - lo)], f32, name=f"u{d}", tag=f"u{d}")
               for d, (lo, hi) in enumerate(CH_DMA)]
        t_cons = pool.tile([P, NC_CONST], f32, name="cons", tag="cons")
        t_scr = pool.tile([P, 8], f32, name="scr", tag="scr")

        def cst(i):
            return t_cons[:, i : i + 1]

        t_a1 = pool.tile([P, W], f32, name="a1", tag="a1")
        t_den = pool.tile([P, W], f32, name="den", tag="den")
        t_rec = pool.tile([P, W], f32, name="rec", tag="rec")
        t_SA = pool.tile([P, W], f32, name="SA", tag="SA")
        t_SC = pool.tile([P, W], f32, name="SC", tag="SC")
        t_SBp = pool.tile([P, W], f32, name="SBp", tag="SBp")
        t_SB = pool.tile([P, W], f32, name="SB", tag="SB")
        t_b = pool.tile([P, W], f32, name="b", tag="b")
        t_c1 = pool.tile([P, W], f32, name="c1", tag="c1")
        t_v = pool.tile([P, W], f32, name="v", tag="v")
        t_a2 = pool.tile([P, W], f32, name="a2", tag="a2")
        t_c2 = pool.tile([P, W], f32, name="c2", tag="c2")
        t_Y0a = pool.tile([P, W], f32, name="Y0a", tag="Y0a")
        t_Y1a = pool.tile([P, W], f32, name="Y1a", tag="Y1a")
        t_Y0b = pool.tile([P, W], f32, name="Y0b", tag="Y0b")
        t_Y1b = pool.tile([P, W], f32, name="Y1b", tag="Y1b")

        def u0q(d):
            lo, hi = CH_DMA[d]
            return t_u[d][:, 0 : 2 * (hi - lo) : 2]

        def u1q(d):
            lo, hi = CH_DMA[d]
            return t_u[d][:, 1 : 2 * (hi - lo) : 2]

        # ---- preamble: DMA issue + engine warms --------------------------
        nc.sync.dma_start(t_cons[:], cons[:])
        # warm scalar DGE queue with a tiny transfer
        nc.scalar.dma_start(t_scr[0:1, 4:8], cons[0:1, 0:4])
        # input chunks: ch0, ch2 on sync queue; ch1, ch3 on scalar queue
        for d, (lo, hi) in enumerate(CH_DMA):
            eng = nc.sync if d % 2 == 0 else nc.scalar
            win = AP(uslab.tensor, 2 * lo, [[L * 2, P], [1, 2 * (hi - lo)]])
            eng.dma_start(t_u[d][:], win)
        # ACT table warm on a Pool-memset scratch (no DMA dependency)
        nc.gpsimd.memset(t_scr[:, 0:4], 0.0)
        nc.scalar.activation(t_scr[:, 0:1], t_scr[:, 1:2], ident,
                             bias=0.0, scale=1.0)
        # Y0-scan additive tile is the constant 1.0 (rescaled states)
        nc.gpsimd.memset(t_b[:], 1.0)
        # Y0a column 0 = i0/c01 (read by c1 chunk 0)
        nc.scalar.activation(t_Y0a[:, 0:1], cst(15), ident, bias=0.0, scale=1.0)

        # ---- op builders -------------------------------------------------
        def act_pre(d):
            lo, hi = CH_DMA[d]
            nc.scalar.activation(t_a1[:, lo:hi], u0q(d), ident,
                                 bias=cst(1), scale=cst(0))
            nc.scalar.activation(t_den[:, lo:hi], u1q(d), ident,
                                 bias=cst(3), scale=cst(2))

        def act_post(d):
            lo, hi = CH_DMA[d]
            nc.scalar.activation(t_SA[:, lo:hi], t_rec[:, lo:hi], ident,
                                 bias=cst(5), scale=cst(4))
            nc.scalar.activation(t_SC[:, lo:hi], u0q(d), ident,
                                 bias=cst(7), scale=cst(6))
            nc.scalar.activation(t_SBp[:, lo:hi], u0q(d), ident,
                                 bias=cst(9), scale=cst(8))

        def rec(d):
            lo, hi = CH_DMA[d]
            nc.vector.reciprocal_approx_fast(t_rec[:, lo:hi], t_den[:, lo:hi])

        def sb(d):  # SBa = SBpa - rec_t   (Pool tensor_tensor subtract)
            lo, hi = CH_DMA[d]
            nc.gpsimd.tensor_tensor(t_SB[:, lo:hi], t_SBp[:, lo:hi],
                                    t_rec[:, lo:hi], op.subtract)

        def c1(d):  # c1 = Y0a + SBa      (Pool tensor_tensor add)
            lo, hi = CH_S1[d]
            nc.gpsimd.tensor_tensor(t_c1[:, lo:hi], t_Y0a[:, lo:hi],
                                    t_SB[:, lo:hi], op.add)

        def c2(e):  # c2 = Y0b + SBa
            lo, hi = CH_S2[e]
            nc.gpsimd.tensor_tensor(t_c2[:, lo:hi], t_Y0b[:, lo:hi],
                                    t_SB[:, lo:hi], op.add)

        def v_(e):  # v = -c02*c01*Y0a + SC   (DVE stt)
            lo, hi = CH_S2[e]
            nc.vector.scalar_tensor_tensor(t_v[:, lo:hi], t_Y0a[:, lo:hi],
                                           cst(13), t_SC[:, lo:hi],
                                           op.mult, op.add)

        def a2_(e):  # a2 = -c03*al*Y1a + v   (DVE stt)
            lo, hi = CH_S2[e]
            nc.vector.scalar_tensor_tensor(t_a2[:, lo:hi], t_Y1a[:, lo:hi],
                                           cst(14), t_v[:, lo:hi],
                                           op.mult, op.add)

        def scanA(d):  # sweep-1 Y0
            lo, hi = CH_S1[d]
            init = cst(15) if d == 0 else t_Y0a[:, lo : lo + 1]
            nc.vector.tensor_tensor_scan(t_Y0a[:, lo + 1 : hi + 1],
                                         t_a1[:, lo:hi], t_b[:, lo:hi],
                                         init, op.mult, op.add)

        def scanB(d):  # sweep-1 Y1
            lo, hi = CH_S1[d]
            init = cst(16) if d == 0 else t_Y1a[:, lo : lo + 1]
            nc.vector.tensor_tensor_scan(t_Y1a[:, lo + 1 : hi + 1],
                                         t_SA[:, lo:hi], t_c1[:, lo:hi],
                                         init, op.mult, op.add)

        def scanC(e):  # sweep-2 Y0
            lo, hi = CH_S2[e]
            init = t_Y0a[:, lo : lo + 1] if e == 0 else t_Y0b[:, lo : lo + 1]
            nc.vector.tensor_tensor_scan(t_Y0b[:, lo + 1 : hi + 1],
                                         t_a2[:, lo:hi], t_b[:, lo:hi],
                                         init, op.mult, op.add)

        def scanD(e):  # sweep-2 Y1
            lo, hi = CH_S2[e]
            init = t_Y1a[:, lo : lo + 1] if e == 0 else t_Y1b[:, lo : lo + 1]
            nc.vector.tensor_tensor_scan(t_Y1b[:, lo + 1 : hi + 1],
                                         t_SA[:, lo:hi], t_c2[:, lo:hi],
                                         init, op.mult, op.add)

        def out0(i):
            lo, hi = OUT_CH[i]
            nc.sync.dma_start(o0[:, lo - K : hi - K], t_Y0b[:, lo:hi])

        def out1(i):
            lo, hi = OUT_CH[i]
            nc.scalar.dma_start(o1[:, lo - K : hi - K], t_Y1b[:, lo:hi])

        def copy64():  # Y0b col 64 = Y0a col 64 (read by c2 chunk 0)
            nc.scalar.activation(t_Y0b[:, J0 : J0 + 1], t_Y0a[:, J0 : J0 + 1],
                                 ident, bias=0.0, scale=1.0)

        # ---- pipelined emission ------------------------------------------
        act_pre(0)
        rec(0)
        act_post(0)
        act_pre(1)
        sb(0)
        scanA(0)
        rec(1)
        act_post(1)
        c1(0)
        scanA(1)
        act_pre(2)
        sb(1)
        scanB(0)
        rec(2)
        act_post(2)
        copy64()
        c1(1)
        scanA(2)
        act_pre(3)
        sb(2)
        scanB(1)
        rec(3)
        act_post(3)
        c1(2)
        scanA(3)
        sb(3)
        scanB(2)
        c1(3)
        v_(0)
        a2_(0)
        scanC(0)
        scanB(3)
        c2(0)
        v_(1)
        a2_(1)
        scanC(1)
        scanD(0)
        c2(1)
        out0(0)
        v_(2)
        a2_(2)
        scanC(2)
        scanD(1)
        c2(2)
        out0(1)
        out1(0)
        v_(3)
        a2_(3)
        scanC(3)
        scanD(2)
        c2(3)
        out0(2)
        out1(1)
        scanD(3)
        out1(2)

    nc.compile()
    _cache["nc"] = nc
    return nc


def _derive(params, x0):
    M, Cc, UA2, Cp, lam, lams, F1, X1p, F3, T1, T200 = [float(params[i]) for i in range(11)]
    UA1 = H * (F1 + F3)
    k1 = (UA1 + F1 * Cp) / lam
    p_ = k1 * B
    q_ = k1 * A
    alpha_u = UA1 * F_ / lam
    alpha_c = (UA1 * G + F1 * Cp * T1) / lam - k1 * C_
    c01 = F1 * X1p / M
    c02 = p_ / M
    c03 = q_ / M
    a10 = -p_ / Cc
    cA2 = -D / (lam * Cc)
    cA1 = 1.0 - q_ / Cc
    cB2 = alpha_u / Cc
    cB1 = alpha_c / Cc
    cB3 = -(E - T200) / (lam * Cc)
    cC2 = alpha_u / M
    cC1 = 1.0 - (F1 - alpha_c) / M
    i0, i1 = float(x0[0]), float(x0[1])
    al = a10 * c01                 # alpha (< 0)
    s_ = -cB3 * UA2 * UA2          # > 0

    cv = np.zeros(NC_CONST, np.float64)
    cv[0] = cC2                           # a1 scale
    cv[1] = cC1 - (c02 * i0 + c03 * i1)   # a1 bias
    cv[2] = 2.0 * Cp * (-al) / s_         # den_t scale
    cv[3] = UA2 * (-al) / s_              # den_t bias
    cv[4] = cA2 * UA2 * UA2 * al / s_     # SA scale (of rec_t)
    cv[5] = cA1 + cA2 * UA2               # SA bias
    cv[6] = cC2                           # SC scale
    cv[7] = cC1                           # SC bias
    cv[8] = cB2 / al                      # SBpa scale
    cv[9] = (cB1 + cB3 * UA2) / al        # SBpa bias
    cv[13] = -c02 * c01                   # v scalar
    cv[14] = -c03 * al                    # a2 scalar
    cv[15] = i0 / c01
    cv[16] = i1 / al
    return cv.astype(np.float32), np.float32(c01), np.float32(al)


def _make_in_maps(u, cons):
    u = np.ascontiguousarray(u, np.float32)
    cons = np.tile(cons[None, :], (P, 1))
    in_maps = []
    for c in range(NCORES):
        if c == 0:
            slab = np.concatenate([np.repeat(u[0:1], K, axis=0), u[0:TC]], axis=0)
        else:
            slab = u[c * TC - K : c * TC + TC]
        in_maps.append({
            "uslab": np.ascontiguousarray(slab),
            "cons": cons,
        })
    return in_maps


def _host_head(u, x0, params, n):
    # exact fp32 simulation of the first n steps (segment 0 has no spin-up)
    f = np.float32
    M, Cc, UA2, Cp, lam, lams, F1, X1p, F3, T1, T200 = [f(params[i]) for i in range(11)]
    out = np.empty((n, 2), f)
    s0, s1 = f(x0[0]), f(x0[1])
    fA, fB, fC, fD, fE, fF, fG, fH = f(A), f(B), f(C_), f(D), f(E), f(F_), f(G), f(H)
    one, two = f(1.0), f(2.0)
    UA1 = fH * (F1 + F3)
    for t in range(n):
        out[t, 0] = s0
        out[t, 1] = s1
        u0, u1 = f(u[t, 0]), f(u[t, 1])
        T2 = fA * s1 + fB * s0 + fC
        T3 = fD * s1 + fE
        T100 = fF * u0 + fG
        Q100 = UA1 * (T100 - T2)
        Q200 = UA2 * (T3 - T200) / (one + UA2 / (two * Cp * u1))
        F5 = Q200 / lam
        F4 = (Q100 - F1 * Cp * (T2 - T1)) / lam
        F2 = F1 - F4
        X2d = (F1 * X1p - F2 * s0) / M
        P2d = (F4 - F5) / Cc
        s0 = s0 + X2d
        s1 = s1 + P2d
    return out


def _assemble(results, head, c01, al):
    out = np.empty((T, 2), np.float32)
    for c in range(NCORES):
        out[c * TC : (c + 1) * TC, 0] = results[c]["o0"].reshape(-1) * c01
        out[c * TC : (c + 1) * TC, 1] = results[c]["o1"].reshape(-1) * al
    out[0:L] = head
    return out


def run(u_forced, x0, params, trace=False):
    from concourse.bass_utils import run_bass_kernel_spmd
    nc = _build_nc()
    cons, c01, al = _derive(params, x0)
    in_maps = _make_in_maps(u_forced, cons)
    head = _host_head(u_forced, x0, params, L)
    res = run_bass_kernel_spmd(nc, in_maps, list(range(NCORES)), trace=trace)
    return _assemble(res.results, head, c01, al), res


def kernel(u_forced, x0, params):
    out, _ = run(u_forced, x0, params, trace=False)
    return out
